# revision 28
# baseline (speedup 1.0000x reference)
"""Bass/Trainium2 kernel for nn_MaskedLoss (MSE with bbox-ROI weighting).

Self-contained: hardcodes shapes (4,1,160,160,160) f32/i32, shards across
8 NeuronCores as (batch item, D-half) pairs, runs one SPMD Bass program
with one tiny pairwise AllReduce for the bbox exchange, and combines the
per-core partial sums on the host.

v9 design (replaces the v3 cumsum-extract scheme):
  - DMA: all 10 mask tiles stream FIRST (alternating the two HWDGE
    queues, 10 SBUF buffers so nothing ring-gates) so the bbox
    collective fires as early as possible; the 20 y_pred/y_true tiles
    follow on the compute-free sync queue, gated on mask completion.
  - mask phase: ACT casts i32->bf16 with per-tile accum (d-sums); PE
    accumulates the 16 h-group column sums (5x320 psum chunks) AND the
    w column sums in PSUM, so projections need no strided DVE reduces.
  - extrema: BIG-trick per PARTITION; the per-axis max/min reduces
    write straight into the [16,8] CC payload (cols ra_h,ra_d,ra_w,
    rb_h,rb_d,rb_w). One pairwise AllReduce(max) exchanges it.
  - post-CC (kept minimal, it sits in the skew-exposed tail): one
    gpsimd partition_all_reduce collapses the 16 payload rows; the
    box bounds for all 3 axes come from a 7-op ladder linear in
    (A,B)=(BIG-mn, BIG+mx); one partition_broadcast fans the 6 bounds
    to 128 partitions; ind128/lhs16 weights follow directly.
  - bulk phase per tile: DVE subtract (f32, in place), ACT Square ->
    bf16 sq tile + f32 accum column (total sums). No scans. The box
    sums come from PE: psum_box[16, (j w)] += lhsT_t^T @ sq_t where
    lhsT_t[p,m] = (p%16==m) * in_dbox(t*8 + p//16) folds the d-axis
    box test into the matmul weights. sq tiles persist in SBUF until
    their matmul runs, so only the cheap PE work waits on the CC.
  - final: box = sum over [16,(j w)] of psum_box * (in_h x in_w) read
    straight from PSUM, totals from the accum columns, both collapsed
    by one ones-matmul; host combines the 8 [2]-vectors.
"""

import os
import sys

import numpy as np

sys.path.insert(0, "/opt/trn_rl_repo")

B, D, H, W = 4, 160, 160, 160
HALF_D = D // 2          # 80 d-slices per core
R = HALF_D * H           # 12800 rows (d,h) per core
KJ = 10                  # rows per partition line in a tile (6400B lines)
NT = R // (128 * KJ)     # 10 tiles per tensor per core
N_CORES = 8
BIG = 1.0e6
W_OUT2 = 0.01            # W_OUT ** 2
EXPAND = 1.2
F = KJ * W               # 1600 free elements per tile partition
CHUNK = 400              # box matmul chunks (4 per tile)
MCHUNK = 320             # mask-phase psum chunks (5 per tile, 2 j-rows each)
SPLIT = 5                # work item after which CC-dependent ops are emitted

_CACHE: dict = {}


def _build_nc():
    from concourse import bacc, bass, bass_isa, tile
    import concourse.mybir as mybir

    f32 = mybir.dt.float32
    i32 = mybir.dt.int32
    bf16 = mybir.dt.bfloat16
    AX = mybir.AxisListType
    OP = mybir.AluOpType
    AF = mybir.ActivationFunctionType
    RO = bass_isa.ReduceOp

    nc = bacc.Bacc(
        "TRN2", target_bir_lowering=False, debug=False, num_devices=N_CORES
    )

    yp = nc.dram_tensor("yp", [R, W], f32, kind="ExternalInput")
    yt = nc.dram_tensor("yt", [R, W], f32, kind="ExternalInput")
    mk = nc.dram_tensor("mk", [R, W], i32, kind="ExternalInput")
    meta = nc.dram_tensor("meta", [1], f32, kind="ExternalInput")
    out = nc.dram_tensor("out", [2], f32, kind="ExternalOutput")

    ypv = yp.ap().rearrange("(t p j) w -> t p j w", p=128, j=KJ)
    ytv = yt.ap().rearrange("(t p j) w -> t p j w", p=128, j=KJ)
    mkv = mk.ap().rearrange("(t p j) w -> t p j w", p=128, j=KJ)

    with tile.TileContext(nc) as tc:
        with (
            tc.tile_pool(name="dram", bufs=1, space="DRAM") as dpool,
            tc.tile_pool(name="persist", bufs=1) as pp,
            tc.tile_pool(name="mkp", bufs=10) as mkp,
            tc.tile_pool(name="mbp", bufs=3) as mbp,
            tc.tile_pool(name="pp2", bufs=6) as ppool,
            tc.tile_pool(name="tp2", bufs=6) as tpool,
            tc.tile_pool(name="psp", bufs=1,
                         space=bass.MemorySpace.PSUM) as pspool,
            tc.tile_pool(name="sqp", bufs=1) as sqpool,
        ):
            cc1_in = dpool.tile([128], f32, tag="cc1_in")
            cc1_out = dpool.tile([128], f32, tag="cc1_out")

            from concourse.tile_rust import add_dep_helper

            # ---------------- mask DMAs first: earliest possible CC ------
            mask_dmas = []
            m_tiles = []
            for t in range(NT):
                m_t = mkp.tile([128, F], i32, tag="m_t")
                if t % 2 == 0:
                    dma = nc.sync.dma_start(out=m_t[:], in_=mkv[t])
                else:
                    dma = nc.scalar.dma_start(out=m_t[:], in_=mkv[t])
                mask_dmas.append(dma)
                m_tiles.append(m_t)
            mask_sync_last = mask_dmas[NT - 2]
            mask_scal_last = mask_dmas[NT - 1]

            # warm the ACT table while mask tile 0 is in flight
            dum = pp.tile([1, 1], f32, tag="dum")
            nc.vector.memset(dum[:], 0.0)
            dum2 = pp.tile([1, 1], f32, tag="dum2")
            nc.scalar.activation(out=dum2[:], in_=dum[:], func=AF.Square)

            # ---------------- setup: iotas / one-hot weights -------------
            # w16b [128,16] bf16 one-hot of p%16; w16f f32 copy for scaling
            a_h = pp.tile([128, 16], i32, tag="a_h")
            nc.gpsimd.iota(a_h[:], pattern=[[-1, 16]], base=0,
                           channel_multiplier=1)          # p - m
            a_h_m = pp.tile([128, 16], i32, tag="a_h_m")
            nc.vector.tensor_scalar(out=a_h_m[:], in0=a_h[:], scalar1=15,
                                    scalar2=None, op0=OP.bitwise_and)
            w16b = pp.tile([128, 16], bf16, tag="w16b")
            nc.vector.tensor_scalar(out=w16b[:], in0=a_h_m[:], scalar1=0,
                                    scalar2=None, op0=OP.is_equal)
            w16f = pp.tile([128, 16], f32, tag="w16f")
            nc.vector.tensor_scalar(out=w16f[:], in0=a_h_m[:], scalar1=0,
                                    scalar2=None, op0=OP.is_equal)
            # w8d [128,8] f32: one-hot of p//16 (for s_d matmul)
            a_d = pp.tile([128, 8], i32, tag="a_d")
            nc.gpsimd.iota(a_d[:], pattern=[[-16, 8]], base=0,
                           channel_multiplier=1)          # p - 16m
            ts1 = pp.tile([128, 8], f32, tag="ts1")
            nc.vector.tensor_scalar(out=ts1[:], in0=a_d[:], scalar1=-1,
                                    scalar2=None, op0=OP.is_gt)
            ts2 = pp.tile([128, 8], f32, tag="ts2")
            nc.vector.tensor_scalar(out=ts2[:], in0=a_d[:], scalar1=15,
                                    scalar2=None, op0=OP.is_le)
            w8d = pp.tile([128, 8], f32, tag="w8d")
            nc.vector.tensor_tensor(out=w8d[:], in0=ts1[:], in1=ts2[:],
                                    op=OP.mult)
            # e_d [8,128] f32: one-hot q == p//16 (expand [8,x] -> [128,x])
            e_d_i = pp.tile([8, 128], i32, tag="e_d_i")
            nc.gpsimd.iota(e_d_i[:], pattern=[[-1, 128]], base=0,
                           channel_multiplier=16)         # 16q - p
            td1 = pp.tile([8, 128], f32, tag="td1")
            nc.vector.tensor_scalar(out=td1[:], in0=e_d_i[:], scalar1=-16,
                                    scalar2=None, op0=OP.is_gt)
            td2 = pp.tile([8, 128], f32, tag="td2")
            nc.vector.tensor_scalar(out=td2[:], in0=e_d_i[:], scalar1=0,
                                    scalar2=None, op0=OP.is_le)
            e_d = pp.tile([8, 128], f32, tag="e_d")
            nc.vector.tensor_tensor(out=e_d[:], in0=td1[:], in1=td2[:],
                                    op=OP.mult)

            ones128b = pp.tile([128, 1], bf16, tag="ones128b")
            nc.vector.memset(ones128b[:], 1.0)
            ones128f = pp.tile([128, 1], f32, tag="ones128f")
            nc.vector.memset(ones128f[:], 1.0)
            ones1x16 = pp.tile([1, 16], f32, tag="ones1x16")
            nc.vector.memset(ones1x16[:], 1.0)

            # identities for PE transposes
            i16_i = pp.tile([16, 16], i32, tag="i16_i")
            nc.gpsimd.iota(i16_i[:], pattern=[[-1, 16]], base=0,
                           channel_multiplier=1)
            ident16 = pp.tile([16, 16], f32, tag="ident16")
            nc.vector.tensor_scalar(out=ident16[:], in0=i16_i[:], scalar1=0,
                                    scalar2=None, op0=OP.is_equal)
            ident8 = ident16[0:8, 0:8]

            # index rows: w [1,160], h [16,10], d [8,10]
            iota_w = pp.tile([1, W], i32, tag="iota_w")
            nc.gpsimd.iota(iota_w[:], pattern=[[1, W]], base=0,
                           channel_multiplier=0)
            k160 = pp.tile([1, W], f32, tag="k160")
            nc.vector.tensor_copy(out=k160[:], in_=iota_w[:])
            iota_h_i = pp.tile([16, KJ], i32, tag="iota_h_i")
            nc.gpsimd.iota(iota_h_i[:], pattern=[[1, KJ]], base=0,
                           channel_multiplier=KJ)         # h = 10q + j
            kh = pp.tile([16, KJ], f32, tag="kh")
            nc.vector.tensor_copy(out=kh[:], in_=iota_h_i[:])
            iota_d_i = pp.tile([8, NT], i32, tag="iota_d_i")
            nc.gpsimd.iota(iota_d_i[:], pattern=[[8, NT]], base=0,
                           channel_multiplier=1)          # d_loc = 8t + q
            meta_s = pp.tile([1, 1], f32, tag="meta_s")
            nc.gpsimd.dma_start(
                out=meta_s[:], in_=meta.ap().rearrange("(p x) -> p x", p=1))
            meta_b8 = pp.tile([8, 1], f32, tag="meta_b8")
            nc.gpsimd.partition_broadcast(meta_b8[:], meta_s[:], channels=8)
            kd = pp.tile([8, NT], f32, tag="kd")
            nc.vector.tensor_copy(out=kd[:], in_=iota_d_i[:])
            nc.vector.tensor_scalar(out=kd[:], in0=kd[:], scalar1=meta_b8[:],
                                    scalar2=None, op0=OP.add)  # global d

            ones1x128 = pp.tile([1, 128], f32, tag="ones1x128")
            nc.vector.memset(ones1x128[:], 1.0)
            pt0 = pspool.tile([128, 512], f32, tag="pmisc", bufs=2)
            nc.tensor.matmul(pt0[:128, :NT], e_d[:], kd[:])
            kd128 = pp.tile([128, NT], f32, tag="kd128")
            nc.vector.tensor_copy(out=kd128[:], in_=pt0[0:128, 0:NT])

            acc_tot = pp.tile([128, 12], f32, tag="acc_tot")
            nc.vector.memset(acc_tot[:], 0.0)

            # precomputed BIG-trick index encodings (off the pre-CC path)
            def big_pair(idx, p, n, tagp):
                bm = pp.tile([p, n], f32, tag=f"bm_{tagp}")
                nc.vector.tensor_scalar(out=bm[:], in0=idx, scalar1=-1.0,
                                        scalar2=BIG, op0=OP.mult, op1=OP.add)
                kp = pp.tile([p, n], f32, tag=f"kp_{tagp}")
                nc.vector.tensor_scalar(out=kp[:], in0=idx, scalar1=BIG,
                                        scalar2=None, op0=OP.add)
                return bm, kp

            bm_w, kp_w = big_pair(k160[:], 1, W, "w")
            bm_h, kp_h = big_pair(kh[:], 16, KJ, "h")
            bm_d, kp_d = big_pair(kd[:], 8, NT, "d")
            big_tbl = {"w": (bm_w, kp_w), "h": (bm_h, kp_h), "d": (bm_d, kp_d)}

            # ---------------- phase 1: mask projections -----------------
            tilesum = pp.tile([128, NT], f32, tag="tilesum")
            psum_h = [pspool.tile([128, 512], f32, tag=f"ph{c}",
                                  name=f"psum_h{c}")
                      for c in range(5)]
            psum_w = pspool.tile([128, 512], f32, tag="pw")

            for t in range(NT):
                m_t = m_tiles[t]
                mb_t = mbp.tile([128, F], bf16, tag="mb_t")
                # cast to bf16 (0/1 exact); accum gives per-(p,t) sums,
                # i.e. d-axis sums since all rows of one (p,t) share d
                nc.scalar.activation(out=mb_t[:], in_=m_t[:], func=AF.Copy,
                                     accum_out=tilesum[:, t : t + 1])
                for c in range(5):
                    nc.tensor.matmul(
                        psum_h[c][:16, :MCHUNK], w16b[:],
                        mb_t[:, c * MCHUNK : (c + 1) * MCHUNK],
                        start=(t == 0), stop=(t == NT - 1))
                for j in range(KJ):
                    nc.tensor.matmul(
                        psum_w[:1, :W], ones128b[:],
                        mb_t[:, j * W : (j + 1) * W],
                        start=(t == 0 and j == 0),
                        stop=(t == NT - 1 and j == KJ - 1))

            # h-group column sums [16, (j w)] -> s_h [16,10]; v_w [1,160]
            s_h = pp.tile([16, KJ], f32, tag="s_h")
            for c in range(5):
                nc.vector.tensor_reduce(
                    out=s_h[:, 2 * c : 2 * c + 2],
                    in_=psum_h[c][0:16, 0:MCHUNK].rearrange(
                        "m (j w) -> m j w", j=2),
                    axis=AX.X, op=OP.add)
            ps_d24 = pspool.tile([128, 512], f32, tag="pmisc", bufs=2)
            nc.tensor.matmul(ps_d24[:8, :NT], w8d[:], tilesum[:])

            # ---------------- extrema (BIG trick, per partition) ---------
            # payload [16,8] cols: 0=ra_h 1=ra_d(r0:8) 2=ra_w(r0) 3=rb_h
            # 4=rb_d(r0:8) 5=rb_w(r0); reduce outs write pk slots directly
            p8w = pp.tile([16, 8], f32, tag="p8w")
            nc.vector.memset(p8w[:], 0.0)

            def extrema(val, p, tagp, slot_a, slot_b, rows):
                n = val.shape[1]
                bm, kp = big_tbl[tagp]
                gt = pp.tile([p, n], f32, tag=f"gt_{tagp}")
                nc.vector.tensor_scalar(out=gt[:], in0=val, scalar1=0.0,
                                        scalar2=None, op0=OP.is_gt)
                ta = pp.tile([p, n], f32, tag=f"ta_{tagp}")
                nc.vector.tensor_tensor(out=ta[:], in0=gt[:], in1=bm[:],
                                        op=OP.mult)
                nc.vector.tensor_reduce(
                    out=p8w[0:rows, slot_a : slot_a + 1], in_=ta[:],
                    axis=AX.X, op=OP.max)
                tb = pp.tile([p, n], f32, tag=f"tb_{tagp}")
                nc.vector.tensor_tensor(out=tb[:], in0=gt[:], in1=kp[:],
                                        op=OP.mult)
                nc.vector.tensor_reduce(
                    out=p8w[0:rows, slot_b : slot_b + 1], in_=tb[:],
                    axis=AX.X, op=OP.max)

            extrema(s_h[:], 16, "h", 0, 3, 16)
            extrema(ps_d24[0:8, 0:NT], 8, "d", 1, 4, 8)
            extrema(psum_w[0:1, 0:W], 1, "w", 2, 5, 1)
            nc.scalar.dma_start(
                out=cc1_in[:].rearrange("(p x) -> p x", p=16), in_=p8w[:])
            nc.gpsimd.collective_compute(
                "AllReduce", OP.max,
                replica_groups=[[0, 1], [2, 3], [4, 5], [6, 7]],
                ins=[cc1_in[:].opt()], outs=[cc1_out[:].opt()])
            g16 = pp.tile([16, 8], f32, tag="g16")
            nc.gpsimd.dma_start(
                out=g16[:], in_=cc1_out[:].rearrange("(p x) -> p x", p=16))

            # ---------------- bulk DMA issue pass ------------------------
            bulk_tiles = []
            for t in range(NT):
                p_t = ppool.tile([128, F], f32, tag="p_t")
                yp_dma = nc.sync.dma_start(out=p_t[:], in_=ypv[t])
                t_t = tpool.tile([128, F], f32, tag="t_t")
                yt_dma = nc.sync.dma_start(out=t_t[:], in_=ytv[t])
                if t == 0:
                    add_dep_helper(yp_dma.ins, mask_sync_last.ins, sync=False,
                                   reason="mask first on sync queue")
                    add_dep_helper(yp_dma.ins, mask_scal_last.ins, sync=True,
                                   reason="mask first (cross queue)")
                bulk_tiles.append((p_t, t_t))

            # ---------------- phase 2: subtract + square -----------------
            work = [(t, 0, KJ) for t in range(NT - 1)]
            work.append((NT - 1, 0, KJ // 2))
            work.append((NT - 1, KJ // 2, KJ))

            box_stop = {0: 9, 1: 9, 2: 10, 3: 10}  # chunk -> last item
            lhs16 = []  # per-tile box matmul weights, built post-CC
            weight16 = pp.tile([16, F], f32, tag="weight16")
            box_psum = [pspool.tile([128, 512], f32, tag=f"ph{c}",
                                    name=f"box_psum{c}")
                        for c in range(4)]

            def emit_cc_dependent():
                # --- cross-partition max on gpsimd (no PE round trips) ---
                # g16 cols: 0=ra_h 1=ra_d 2=ra_w 3=rb_h 4=rb_d 5=rb_w
                gmax = pp.tile([16, 8], f32, tag="gmax")
                nc.gpsimd.partition_all_reduce(gmax[:], g16[:], channels=16,
                                               reduce_op=RO.max)
                t8 = gmax[0:1, :]

                # --- bounds, all 3 axes at once (linear in A=t8[0:3],
                # B=t8[3:6]):  lo_cmp = -1.1A - 0.1B + 1.2*BIG - 1.6
                #              hi_cmp = min(0.1A + 1.1B - 1.2*BIG - 0.4, 158)
                lh = pp.tile([1, 8], f32, tag="lh")
                u3 = pp.tile([1, 3], f32, tag="u3")
                nc.vector.tensor_scalar(out=u3[:], in0=t8[:, 0:3],
                                        scalar1=-1.1, scalar2=1.2 * BIG - 1.6,
                                        op0=OP.mult, op1=OP.add)
                v3 = pp.tile([1, 3], f32, tag="v3")
                nc.vector.tensor_scalar(out=v3[:], in0=t8[:, 3:6],
                                        scalar1=0.1, scalar2=None,
                                        op0=OP.mult)
                nc.vector.tensor_tensor(out=lh[:, 0:3], in0=u3[:], in1=v3[:],
                                        op=OP.subtract)
                w3 = pp.tile([1, 3], f32, tag="w3")
                nc.vector.tensor_scalar(out=w3[:], in0=t8[:, 0:3],
                                        scalar1=0.1,
                                        scalar2=-1.2 * BIG - 0.4,
                                        op0=OP.mult, op1=OP.add)
                x3 = pp.tile([1, 3], f32, tag="x3")
                nc.vector.tensor_scalar(out=x3[:], in0=t8[:, 3:6],
                                        scalar1=1.1, scalar2=None,
                                        op0=OP.mult)
                nc.vector.tensor_tensor(out=lh[:, 3:6], in0=w3[:], in1=x3[:],
                                        op=OP.add)
                nc.vector.tensor_scalar(out=lh[:, 3:6], in0=lh[:, 3:6],
                                        scalar1=float(W - 2), scalar2=None,
                                        op0=OP.min)
                hf = pp.tile([1, 1], f32, tag="hf")
                nc.vector.tensor_scalar(out=hf[:], in0=t8[:, 2:3],
                                        scalar1=0.0, scalar2=None,
                                        op0=OP.is_gt)

                # broadcast all 6 bounds to 128 partitions on gpsimd
                bc128 = pp.tile([128, 6], f32, tag="bc128")
                nc.gpsimd.partition_broadcast(bc128[:], lh[0:1, 0:6],
                                              channels=128)

                # ind128 [128,NT] in-box(d) per (p//16, t); in_h [16,KJ]
                ga128 = pp.tile([128, NT], f32, tag="ga128")
                nc.vector.tensor_scalar(out=ga128[:], in0=kd128[:],
                                        scalar1=bc128[:, 1:2],
                                        scalar2=None, op0=OP.is_gt)
                ind128 = pp.tile([128, NT], f32, tag="ind128")
                nc.vector.tensor_scalar(out=ind128[:], in0=kd128[:],
                                        scalar1=bc128[:, 4:5],
                                        scalar2=None, op0=OP.is_le)
                nc.vector.tensor_tensor(out=ind128[:], in0=ind128[:],
                                        in1=ga128[:], op=OP.mult)
                for t in range(NT):
                    lt = pp.tile([128, 16], bf16, tag=f"lhs16_{t}")
                    nc.vector.tensor_scalar(
                        out=lt[:], in0=w16f[:],
                        scalar1=ind128[:, t : t + 1], scalar2=None,
                        op0=OP.mult)
                    lhs16.append(lt)

                ga_h = pp.tile([16, KJ], f32, tag="ga_h")
                nc.vector.tensor_scalar(out=ga_h[:], in0=kh[:],
                                        scalar1=bc128[0:16, 0:1],
                                        scalar2=None, op0=OP.is_gt)
                in_h = pp.tile([16, KJ], f32, tag="in_h")
                nc.vector.tensor_scalar(out=in_h[:], in0=kh[:],
                                        scalar1=bc128[0:16, 3:4],
                                        scalar2=None, op0=OP.is_le)
                nc.vector.tensor_tensor(out=in_h[:], in0=in_h[:], in1=ga_h[:],
                                        op=OP.mult)

                # wrow [1,W] (hasfg folded in) -> wrow16 [16,W]
                gw = pp.tile([1, W], f32, tag="gw")
                nc.vector.tensor_scalar(out=gw[:], in0=k160[:],
                                        scalar1=bc128[0:1, 2:3], scalar2=None,
                                        op0=OP.is_gt)
                wrow = pp.tile([1, W], f32, tag="wrow")
                nc.vector.tensor_scalar(out=wrow[:], in0=k160[:],
                                        scalar1=bc128[0:1, 5:6], scalar2=None,
                                        op0=OP.is_le)
                nc.vector.tensor_tensor(out=wrow[:], in0=wrow[:], in1=gw[:],
                                        op=OP.mult)
                nc.vector.tensor_scalar(out=wrow[:], in0=wrow[:],
                                        scalar1=hf[:], scalar2=None,
                                        op0=OP.mult)
                pt4 = pspool.tile([128, 512], f32, tag="pmisc", bufs=2)
                nc.tensor.matmul(pt4[:16, :W], ones1x16[:], wrow[:])
                wrow16 = pp.tile([16, W], f32, tag="wrow16")
                nc.vector.tensor_copy(out=wrow16[:], in_=pt4[0:16, 0:W])
                # weight16 [16,(j w)] = in_h[:,j] * wrow  (bf16)
                for j in range(KJ):
                    nc.vector.tensor_scalar(
                        out=weight16[:, j * W : (j + 1) * W], in0=wrow16[:],
                        scalar1=in_h[:, j : j + 1], scalar2=None, op0=OP.mult)

            def emit_box_mm(i):
                t, j0, j1 = work[i]
                nj = j1 - j0
                sq_i = sq_tiles[i]
                for cl in range((nj * W) // CHUNK):
                    c = (j0 * W) // CHUNK + cl
                    nc.tensor.matmul(
                        box_psum[c][:16, :CHUNK], lhs16[t][:],
                        sq_i[:, cl * CHUNK : (cl + 1) * CHUNK],
                        start=(i == 0), stop=(i == box_stop[c]))

            sq_tiles = []
            for i, (t, j0, j1) in enumerate(work):
                p_t, t_t = bulk_tiles[t]
                fsl = slice(j0 * W, j1 * W)
                nc.vector.tensor_tensor(out=p_t[:, fsl], in0=p_t[:, fsl],
                                        in1=t_t[:, fsl], op=OP.subtract)
                sq_i = sqpool.tile([128, F], bf16, tag=f"sq_{i}")
                nc.scalar.activation(
                    out=sq_i[:, : (j1 - j0) * W], in_=p_t[:, fsl],
                    func=AF.Square, accum_out=acc_tot[:, i : i + 1])
                sq_tiles.append(sq_i)
                if i > SPLIT:
                    emit_box_mm(i)
                if i == SPLIT:
                    emit_cc_dependent()
                    for ii in range(SPLIT + 1):
                        emit_box_mm(ii)

            # ---------------- final reductions ----------------
            junk16 = pp.tile([16, F], bf16, tag="junk16")
            for c in range(4):
                nc.vector.tensor_tensor(
                    out=junk16[:, c * CHUNK : (c + 1) * CHUNK],
                    in0=box_psum[c][0:16, 0:CHUNK],
                    in1=weight16[:, c * CHUNK : (c + 1) * CHUNK],
                    op=OP.mult)
            box_col = pp.tile([16, 1], f32, tag="box_col")
            nc.vector.tensor_reduce(out=box_col[:], in_=junk16[:], axis=AX.X,
                                    op=OP.add)
            tot_col = pp.tile([128, 1], f32, tag="tot_col")
            nc.vector.tensor_reduce(out=tot_col[:], in_=acc_tot[:],
                                    axis=AX.X, op=OP.add)
            pair = pp.tile([128, 2], f32, tag="pair")
            nc.vector.memset(pair[:], 0.0)
            nc.vector.tensor_copy(out=pair[:, 0:1], in_=tot_col[:])
            nc.vector.tensor_copy(out=pair[0:16, 1:2], in_=box_col[:])
            ps_fin = pspool.tile([128, 512], f32, tag="pmisc", bufs=2)
            nc.tensor.matmul(ps_fin[:1, :2], ones128f[:], pair[:])
            res2 = pp.tile([1, 2], f32, tag="res2")
            nc.vector.tensor_copy(out=res2[:], in_=ps_fin[0:1, 0:2])
            nc.scalar.dma_start(
                out=out.ap().rearrange("(p x) -> p x", p=1), in_=res2[:])

    nc.compile()
    return nc


def get_nc():
    if "nc" not in _CACHE:
        _CACHE["nc"] = _build_nc()
    return _CACHE["nc"]


def make_in_maps(y_pred, y_true, mask):
    y_pred = np.asarray(y_pred, dtype=np.float32).reshape(B, D, H, W)
    y_true = np.asarray(y_true, dtype=np.float32).reshape(B, D, H, W)
    mask = np.asarray(mask, dtype=np.int32).reshape(B, D, H, W)
    in_maps = []
    for c in range(N_CORES):
        b, half = c // 2, c % 2
        sl = slice(half * HALF_D, (half + 1) * HALF_D)
        in_maps.append({
            "yp": np.ascontiguousarray(y_pred[b, sl]).reshape(R, W),
            "yt": np.ascontiguousarray(y_true[b, sl]).reshape(R, W),
            "mk": np.ascontiguousarray(mask[b, sl]).reshape(R, W),
            "meta": np.array([half * HALF_D], dtype=np.float32),
        })
    return in_maps


def combine(results):
    tot = 0.0
    box = 0.0
    for r in results:
        o = np.asarray(r["out"], dtype=np.float64).reshape(-1)
        tot += o[0]
        box += o[1]
    loss = (W_OUT2 * tot + (1.0 - W_OUT2) * box) / float(B * D * H * W)
    return np.array(loss, dtype=np.float32)


def kernel(y_pred, y_true, mask):
    from concourse.bass_utils import run_bass_kernel_spmd

    nc = get_nc()
    in_maps = make_in_maps(y_pred, y_true, mask)
    trace = bool(int(os.environ.get("BASS_KERNEL_TRACE", "0")))
    kwargs = {}
    if trace:
        kwargs = dict(trace=True, trace_cores=[0])
    res = run_bass_kernel_spmd(
        nc, in_maps, core_ids=list(range(N_CORES)), **kwargs
    )
    _CACHE["last_results"] = res
    return combine(res.results)


# revision 31
# speedup vs baseline: 1.0224x; 1.0224x over previous
"""Bass/Trainium2 kernel for nn_MaskedLoss (MSE with bbox-ROI weighting).

Self-contained: hardcodes shapes (4,1,160,160,160) f32/i32, shards across
8 NeuronCores as (batch item, D-half) pairs, runs one SPMD Bass program
with one tiny pairwise AllReduce for the bbox exchange, and combines the
per-core partial sums on the host.

v9 design (replaces the v3 cumsum-extract scheme):
  - DMA: all 10 mask tiles stream FIRST (alternating the two HWDGE
    queues, 10 SBUF buffers so nothing ring-gates) so the bbox
    collective fires as early as possible; the 20 y_pred/y_true tiles
    follow on the compute-free sync queue, gated on mask completion.
  - mask phase: ACT casts i32->bf16 with per-tile accum (d-sums); PE
    accumulates the 16 h-group column sums (5x320 psum chunks) AND the
    w column sums in PSUM, so projections need no strided DVE reduces.
  - extrema: BIG-trick per PARTITION; the per-axis max/min reduces
    write straight into the [16,8] CC payload (cols ra_h,ra_d,ra_w,
    rb_h,rb_d,rb_w). One pairwise AllReduce(max) exchanges it.
  - post-CC (kept minimal, it sits in the skew-exposed tail): one
    gpsimd partition_all_reduce collapses the 16 payload rows; the
    box bounds for all 3 axes come from a 7-op ladder linear in
    (A,B)=(BIG-mn, BIG+mx); one partition_broadcast fans the 6 bounds
    to 128 partitions; ind128/lhs16 weights follow directly.
  - bulk phase per tile: DVE subtract (f32, in place), ACT Square ->
    bf16 sq tile + f32 accum column (total sums). No scans. The box
    sums come from PE: psum_box[16, (j w)] += lhsT_t^T @ sq_t where
    lhsT_t[p,m] = (p%16==m) * in_dbox(t*8 + p//16) folds the d-axis
    box test into the matmul weights. sq tiles persist in SBUF until
    their matmul runs, so only the cheap PE work waits on the CC.
  - final: box = sum over [16,(j w)] of psum_box * (in_h x in_w) read
    straight from PSUM, totals from the accum columns, both collapsed
    by one ones-matmul; host combines the 8 [2]-vectors.
"""

import os
import sys

import numpy as np

sys.path.insert(0, "/opt/trn_rl_repo")

B, D, H, W = 4, 160, 160, 160
HALF_D = D // 2          # 80 d-slices per core
R = HALF_D * H           # 12800 rows (d,h) per core
KJ = 10                  # rows per partition line in a tile (6400B lines)
NT = R // (128 * KJ)     # 10 tiles per tensor per core
N_CORES = 8
BIG = 1.0e6
W_OUT2 = 0.01            # W_OUT ** 2
EXPAND = 1.2
F = KJ * W               # 1600 free elements per tile partition
CHUNK = 400              # box matmul chunks (4 per tile)
MCHUNK = 320             # mask-phase psum chunks (5 per tile, 2 j-rows each)
SPLIT = 5                # work item after which CC-dependent ops are emitted

_CACHE: dict = {}


def _build_nc():
    from concourse import bacc, bass, bass_isa, tile
    import concourse.mybir as mybir

    f32 = mybir.dt.float32
    i32 = mybir.dt.int32
    bf16 = mybir.dt.bfloat16
    AX = mybir.AxisListType
    OP = mybir.AluOpType
    AF = mybir.ActivationFunctionType
    RO = bass_isa.ReduceOp

    nc = bacc.Bacc(
        "TRN2", target_bir_lowering=False, debug=False, num_devices=N_CORES
    )

    yp = nc.dram_tensor("yp", [R, W], f32, kind="ExternalInput")
    yt = nc.dram_tensor("yt", [R, W], f32, kind="ExternalInput")
    mk = nc.dram_tensor("mk", [R, W], i32, kind="ExternalInput")
    meta = nc.dram_tensor("meta", [1], f32, kind="ExternalInput")
    out = nc.dram_tensor("out", [2], f32, kind="ExternalOutput")

    ypv = yp.ap().rearrange("(t p j) w -> t p j w", p=128, j=KJ)
    ytv = yt.ap().rearrange("(t p j) w -> t p j w", p=128, j=KJ)
    mkv = mk.ap().rearrange("(t p j) w -> t p j w", p=128, j=KJ)

    with tile.TileContext(nc) as tc:
        with (
            tc.tile_pool(name="dram", bufs=1, space="DRAM") as dpool,
            tc.tile_pool(name="persist", bufs=1) as pp,
            tc.tile_pool(name="mkp", bufs=10) as mkp,
            tc.tile_pool(name="mbp", bufs=3) as mbp,
            tc.tile_pool(name="pp2", bufs=6) as ppool,
            tc.tile_pool(name="tp2", bufs=6) as tpool,
            tc.tile_pool(name="psp", bufs=1,
                         space=bass.MemorySpace.PSUM) as pspool,
            tc.tile_pool(name="sqp", bufs=1) as sqpool,
        ):
            cc1_in = dpool.tile([128], f32, tag="cc1_in")
            cc1_out = dpool.tile([128], f32, tag="cc1_out")

            from concourse.tile_rust import add_dep_helper

            # ---------------- mask DMAs first: earliest possible CC ------
            mask_dmas = []
            m_tiles = []
            for t in range(NT):
                m_t = mkp.tile([128, F], i32, tag="m_t")
                if t % 2 == 0:
                    dma = nc.sync.dma_start(out=m_t[:], in_=mkv[t])
                else:
                    dma = nc.scalar.dma_start(out=m_t[:], in_=mkv[t])
                mask_dmas.append(dma)
                m_tiles.append(m_t)
            mask_sync_last = mask_dmas[NT - 2]
            mask_scal_last = mask_dmas[NT - 1]

            # warm the ACT table while mask tile 0 is in flight
            dum = pp.tile([1, 1], f32, tag="dum")
            nc.vector.memset(dum[:], 0.0)
            dum2 = pp.tile([1, 1], f32, tag="dum2")
            nc.scalar.activation(out=dum2[:], in_=dum[:], func=AF.Square)

            # ---------------- setup: iotas / one-hot weights -------------
            # w16b [128,16] bf16 one-hot of p%16; w16f f32 copy for scaling
            a_h = pp.tile([128, 16], i32, tag="a_h")
            nc.gpsimd.iota(a_h[:], pattern=[[-1, 16]], base=0,
                           channel_multiplier=1)          # p - m
            a_h_m = pp.tile([128, 16], i32, tag="a_h_m")
            nc.vector.tensor_scalar(out=a_h_m[:], in0=a_h[:], scalar1=15,
                                    scalar2=None, op0=OP.bitwise_and)
            w16b = pp.tile([128, 16], bf16, tag="w16b")
            nc.vector.tensor_scalar(out=w16b[:], in0=a_h_m[:], scalar1=0,
                                    scalar2=None, op0=OP.is_equal)
            w16f = pp.tile([128, 16], f32, tag="w16f")
            nc.vector.tensor_scalar(out=w16f[:], in0=a_h_m[:], scalar1=0,
                                    scalar2=None, op0=OP.is_equal)
            # w8d [128,8] f32: one-hot of p//16 (for s_d matmul)
            a_d = pp.tile([128, 8], i32, tag="a_d")
            nc.gpsimd.iota(a_d[:], pattern=[[-16, 8]], base=0,
                           channel_multiplier=1)          # p - 16m
            ts1 = pp.tile([128, 8], f32, tag="ts1")
            nc.vector.tensor_scalar(out=ts1[:], in0=a_d[:], scalar1=-1,
                                    scalar2=None, op0=OP.is_gt)
            ts2 = pp.tile([128, 8], f32, tag="ts2")
            nc.vector.tensor_scalar(out=ts2[:], in0=a_d[:], scalar1=15,
                                    scalar2=None, op0=OP.is_le)
            w8d = pp.tile([128, 8], f32, tag="w8d")
            nc.vector.tensor_tensor(out=w8d[:], in0=ts1[:], in1=ts2[:],
                                    op=OP.mult)
            # e_d [8,128] f32: one-hot q == p//16 (expand [8,x] -> [128,x])
            e_d_i = pp.tile([8, 128], i32, tag="e_d_i")
            nc.gpsimd.iota(e_d_i[:], pattern=[[-1, 128]], base=0,
                           channel_multiplier=16)         # 16q - p
            td1 = pp.tile([8, 128], f32, tag="td1")
            nc.vector.tensor_scalar(out=td1[:], in0=e_d_i[:], scalar1=-16,
                                    scalar2=None, op0=OP.is_gt)
            td2 = pp.tile([8, 128], f32, tag="td2")
            nc.vector.tensor_scalar(out=td2[:], in0=e_d_i[:], scalar1=0,
                                    scalar2=None, op0=OP.is_le)
            e_d = pp.tile([8, 128], f32, tag="e_d")
            nc.vector.tensor_tensor(out=e_d[:], in0=td1[:], in1=td2[:],
                                    op=OP.mult)

            ones128b = pp.tile([128, 1], bf16, tag="ones128b")
            nc.vector.memset(ones128b[:], 1.0)
            ones128f = pp.tile([128, 1], f32, tag="ones128f")
            nc.vector.memset(ones128f[:], 1.0)
            ones1x16 = pp.tile([1, 16], f32, tag="ones1x16")
            nc.vector.memset(ones1x16[:], 1.0)

            # identities for PE transposes
            i16_i = pp.tile([16, 16], i32, tag="i16_i")
            nc.gpsimd.iota(i16_i[:], pattern=[[-1, 16]], base=0,
                           channel_multiplier=1)
            ident16 = pp.tile([16, 16], f32, tag="ident16")
            nc.vector.tensor_scalar(out=ident16[:], in0=i16_i[:], scalar1=0,
                                    scalar2=None, op0=OP.is_equal)
            ident8 = ident16[0:8, 0:8]

            # index rows: w [1,160], h [16,10], d [8,10]
            iota_w = pp.tile([1, W], i32, tag="iota_w")
            nc.gpsimd.iota(iota_w[:], pattern=[[1, W]], base=0,
                           channel_multiplier=0)
            k160 = pp.tile([1, W], f32, tag="k160")
            nc.vector.tensor_copy(out=k160[:], in_=iota_w[:])
            iota_h_i = pp.tile([16, KJ], i32, tag="iota_h_i")
            nc.gpsimd.iota(iota_h_i[:], pattern=[[1, KJ]], base=0,
                           channel_multiplier=KJ)         # h = 10q + j
            kh = pp.tile([16, KJ], f32, tag="kh")
            nc.vector.tensor_copy(out=kh[:], in_=iota_h_i[:])
            iota_d_i = pp.tile([8, NT], i32, tag="iota_d_i")
            nc.gpsimd.iota(iota_d_i[:], pattern=[[8, NT]], base=0,
                           channel_multiplier=1)          # d_loc = 8t + q
            meta_s = pp.tile([1, 1], f32, tag="meta_s")
            nc.scalar.dma_start(
                out=meta_s[:], in_=meta.ap().rearrange("(p x) -> p x", p=1))
            meta_b8 = pp.tile([8, 1], f32, tag="meta_b8")
            nc.gpsimd.partition_broadcast(meta_b8[:], meta_s[:], channels=8)
            kd = pp.tile([8, NT], f32, tag="kd")
            nc.vector.tensor_copy(out=kd[:], in_=iota_d_i[:])
            nc.vector.tensor_scalar(out=kd[:], in0=kd[:], scalar1=meta_b8[:],
                                    scalar2=None, op0=OP.add)  # global d

            ones1x128 = pp.tile([1, 128], f32, tag="ones1x128")
            nc.vector.memset(ones1x128[:], 1.0)
            pt0 = pspool.tile([128, 512], f32, tag="pmisc", bufs=2)
            nc.tensor.matmul(pt0[:128, :NT], e_d[:], kd[:])
            kd128 = pp.tile([128, NT], f32, tag="kd128")
            nc.vector.tensor_copy(out=kd128[:], in_=pt0[0:128, 0:NT])

            acc_tot = pp.tile([128, 12], f32, tag="acc_tot")
            nc.vector.memset(acc_tot[:], 0.0)

            # precomputed BIG-trick index encodings (off the pre-CC path)
            def big_pair(idx, p, n, tagp):
                bm = pp.tile([p, n], f32, tag=f"bm_{tagp}")
                nc.vector.tensor_scalar(out=bm[:], in0=idx, scalar1=-1.0,
                                        scalar2=BIG, op0=OP.mult, op1=OP.add)
                kp = pp.tile([p, n], f32, tag=f"kp_{tagp}")
                nc.vector.tensor_scalar(out=kp[:], in0=idx, scalar1=BIG,
                                        scalar2=None, op0=OP.add)
                return bm, kp

            bm_w, kp_w = big_pair(k160[:], 1, W, "w")
            bm_h, kp_h = big_pair(kh[:], 16, KJ, "h")
            bm_d, kp_d = big_pair(kd[:], 8, NT, "d")
            big_tbl = {"w": (bm_w, kp_w), "h": (bm_h, kp_h), "d": (bm_d, kp_d)}

            # ---------------- phase 1: mask projections -----------------
            tilesum = pp.tile([128, NT], f32, tag="tilesum")
            psum_h = [pspool.tile([128, 512], f32, tag=f"ph{c}",
                                  name=f"psum_h{c}")
                      for c in range(5)]
            psum_w = pspool.tile([128, 512], f32, tag="pw")

            for t in range(NT):
                m_t = m_tiles[t]
                mb_t = mbp.tile([128, F], bf16, tag="mb_t")
                # cast to bf16 (0/1 exact); accum gives per-(p,t) sums,
                # i.e. d-axis sums since all rows of one (p,t) share d
                nc.scalar.activation(out=mb_t[:], in_=m_t[:], func=AF.Copy,
                                     accum_out=tilesum[:, t : t + 1])
                for c in range(5):
                    nc.tensor.matmul(
                        psum_h[c][:16, :MCHUNK], w16b[:],
                        mb_t[:, c * MCHUNK : (c + 1) * MCHUNK],
                        start=(t == 0), stop=(t == NT - 1))
                for j in range(KJ):
                    nc.tensor.matmul(
                        psum_w[:1, :W], ones128b[:],
                        mb_t[:, j * W : (j + 1) * W],
                        start=(t == 0 and j == 0),
                        stop=(t == NT - 1 and j == KJ - 1))

            # h-group column sums [16, (j w)] -> s_h [16,10]; v_w [1,160]
            s_h = pp.tile([16, KJ], f32, tag="s_h")
            for c in range(5):
                nc.vector.tensor_reduce(
                    out=s_h[:, 2 * c : 2 * c + 2],
                    in_=psum_h[c][0:16, 0:MCHUNK].rearrange(
                        "m (j w) -> m j w", j=2),
                    axis=AX.X, op=OP.add)
            ps_d24 = pspool.tile([128, 512], f32, tag="pmisc", bufs=2)
            nc.tensor.matmul(ps_d24[:8, :NT], w8d[:], tilesum[:])

            # ---------------- extrema (BIG trick, per partition) ---------
            # payload [16,8] cols: 0=ra_h 1=ra_d(r0:8) 2=ra_w(r0) 3=rb_h
            # 4=rb_d(r0:8) 5=rb_w(r0); reduce outs write pk slots directly
            p8w = pp.tile([16, 8], f32, tag="p8w")
            nc.vector.memset(p8w[:], 0.0)

            def extrema(val, p, tagp, slot_a, slot_b, rows):
                n = val.shape[1]
                bm, kp = big_tbl[tagp]
                gt = pp.tile([p, n], f32, tag=f"gt_{tagp}")
                nc.vector.tensor_scalar(out=gt[:], in0=val, scalar1=0.0,
                                        scalar2=None, op0=OP.is_gt)
                ta = pp.tile([p, n], f32, tag=f"ta_{tagp}")
                nc.vector.tensor_tensor(out=ta[:], in0=gt[:], in1=bm[:],
                                        op=OP.mult)
                nc.vector.tensor_reduce(
                    out=p8w[0:rows, slot_a : slot_a + 1], in_=ta[:],
                    axis=AX.X, op=OP.max)
                tb = pp.tile([p, n], f32, tag=f"tb_{tagp}")
                nc.vector.tensor_tensor(out=tb[:], in0=gt[:], in1=kp[:],
                                        op=OP.mult)
                nc.vector.tensor_reduce(
                    out=p8w[0:rows, slot_b : slot_b + 1], in_=tb[:],
                    axis=AX.X, op=OP.max)

            extrema(s_h[:], 16, "h", 0, 3, 16)
            extrema(ps_d24[0:8, 0:NT], 8, "d", 1, 4, 8)
            extrema(psum_w[0:1, 0:W], 1, "w", 2, 5, 1)
            nc.scalar.dma_start(
                out=cc1_in[:].rearrange("(p x) -> p x", p=16), in_=p8w[:])
            nc.gpsimd.collective_compute(
                "AllReduce", OP.max,
                replica_groups=[[0, 1], [2, 3], [4, 5], [6, 7]],
                ins=[cc1_in[:].opt()], outs=[cc1_out[:].opt()])
            g16 = pp.tile([16, 8], f32, tag="g16")
            nc.sync.dma_start(
                out=g16[:], in_=cc1_out[:].rearrange("(p x) -> p x", p=16))

            # ---------------- bulk DMA issue pass ------------------------
            bulk_tiles = []
            for t in range(NT):
                p_t = ppool.tile([128, F], f32, tag="p_t")
                yp_dma = nc.sync.dma_start(out=p_t[:], in_=ypv[t])
                t_t = tpool.tile([128, F], f32, tag="t_t")
                yt_dma = nc.sync.dma_start(out=t_t[:], in_=ytv[t])
                if t == 0:
                    add_dep_helper(yp_dma.ins, mask_sync_last.ins, sync=False,
                                   reason="mask first on sync queue")
                    add_dep_helper(yp_dma.ins, mask_scal_last.ins, sync=True,
                                   reason="mask first (cross queue)")
                bulk_tiles.append((p_t, t_t))

            # ---------------- phase 2: subtract + square -----------------
            work = [(t, 0, KJ) for t in range(NT - 1)]
            work.append((NT - 1, 0, KJ // 2))
            work.append((NT - 1, KJ // 2, KJ))

            box_stop = {0: 9, 1: 9, 2: 10, 3: 10}  # chunk -> last item
            lhs16 = []  # per-tile box matmul weights, built post-CC
            weight16 = pp.tile([16, F], f32, tag="weight16")
            box_psum = [pspool.tile([128, 512], f32, tag=f"ph{c}",
                                    name=f"box_psum{c}")
                        for c in range(4)]

            def emit_cc_dependent():
                # --- cross-partition max on gpsimd (no PE round trips) ---
                # g16 cols: 0=ra_h 1=ra_d 2=ra_w 3=rb_h 4=rb_d 5=rb_w
                gmax = pp.tile([16, 8], f32, tag="gmax")
                nc.gpsimd.partition_all_reduce(gmax[:], g16[:], channels=16,
                                               reduce_op=RO.max)
                t8 = gmax[0:1, :]

                # --- bounds, all 3 axes at once (linear in A=t8[0:3],
                # B=t8[3:6]):  lo_cmp = -1.1A - 0.1B + 1.2*BIG - 1.6
                #              hi_cmp = min(0.1A + 1.1B - 1.2*BIG - 0.4, 158)
                lh = pp.tile([1, 8], f32, tag="lh")
                u3 = pp.tile([1, 3], f32, tag="u3")
                nc.vector.tensor_scalar(out=u3[:], in0=t8[:, 0:3],
                                        scalar1=-1.1, scalar2=1.2 * BIG - 1.6,
                                        op0=OP.mult, op1=OP.add)
                v3 = pp.tile([1, 3], f32, tag="v3")
                nc.vector.tensor_scalar(out=v3[:], in0=t8[:, 3:6],
                                        scalar1=0.1, scalar2=None,
                                        op0=OP.mult)
                nc.vector.tensor_tensor(out=lh[:, 0:3], in0=u3[:], in1=v3[:],
                                        op=OP.subtract)
                w3 = pp.tile([1, 3], f32, tag="w3")
                nc.vector.tensor_scalar(out=w3[:], in0=t8[:, 0:3],
                                        scalar1=0.1,
                                        scalar2=-1.2 * BIG - 0.4,
                                        op0=OP.mult, op1=OP.add)
                x3 = pp.tile([1, 3], f32, tag="x3")
                nc.vector.tensor_scalar(out=x3[:], in0=t8[:, 3:6],
                                        scalar1=1.1, scalar2=None,
                                        op0=OP.mult)
                nc.vector.tensor_tensor(out=lh[:, 3:6], in0=w3[:], in1=x3[:],
                                        op=OP.add)
                nc.vector.tensor_scalar(out=lh[:, 3:6], in0=lh[:, 3:6],
                                        scalar1=float(W - 2), scalar2=None,
                                        op0=OP.min)
                hf = pp.tile([1, 1], f32, tag="hf")
                nc.vector.tensor_scalar(out=hf[:], in0=t8[:, 2:3],
                                        scalar1=0.0, scalar2=None,
                                        op0=OP.is_gt)

                # broadcast all 6 bounds to 128 partitions on gpsimd
                bc128 = pp.tile([128, 6], f32, tag="bc128")
                nc.gpsimd.partition_broadcast(bc128[:], lh[0:1, 0:6],
                                              channels=128)

                # ind128 [128,NT] in-box(d) per (p//16, t); in_h [16,KJ]
                ga128 = pp.tile([128, NT], f32, tag="ga128")
                nc.vector.tensor_scalar(out=ga128[:], in0=kd128[:],
                                        scalar1=bc128[:, 1:2],
                                        scalar2=None, op0=OP.is_gt)
                ind128 = pp.tile([128, NT], f32, tag="ind128")
                nc.vector.tensor_scalar(out=ind128[:], in0=kd128[:],
                                        scalar1=bc128[:, 4:5],
                                        scalar2=None, op0=OP.is_le)
                nc.vector.tensor_tensor(out=ind128[:], in0=ind128[:],
                                        in1=ga128[:], op=OP.mult)
                for t in range(NT):
                    lt = pp.tile([128, 16], bf16, tag=f"lhs16_{t}")
                    nc.vector.tensor_scalar(
                        out=lt[:], in0=w16f[:],
                        scalar1=ind128[:, t : t + 1], scalar2=None,
                        op0=OP.mult)
                    lhs16.append(lt)

                ga_h = pp.tile([16, KJ], f32, tag="ga_h")
                nc.vector.tensor_scalar(out=ga_h[:], in0=kh[:],
                                        scalar1=bc128[0:16, 0:1],
                                        scalar2=None, op0=OP.is_gt)
                in_h = pp.tile([16, KJ], f32, tag="in_h")
                nc.vector.tensor_scalar(out=in_h[:], in0=kh[:],
                                        scalar1=bc128[0:16, 3:4],
                                        scalar2=None, op0=OP.is_le)
                nc.vector.tensor_tensor(out=in_h[:], in0=in_h[:], in1=ga_h[:],
                                        op=OP.mult)

                # wrow [1,W] (hasfg folded in) -> wrow16 [16,W]
                gw = pp.tile([1, W], f32, tag="gw")
                nc.vector.tensor_scalar(out=gw[:], in0=k160[:],
                                        scalar1=bc128[0:1, 2:3], scalar2=None,
                                        op0=OP.is_gt)
                wrow = pp.tile([1, W], f32, tag="wrow")
                nc.vector.tensor_scalar(out=wrow[:], in0=k160[:],
                                        scalar1=bc128[0:1, 5:6], scalar2=None,
                                        op0=OP.is_le)
                nc.vector.tensor_tensor(out=wrow[:], in0=wrow[:], in1=gw[:],
                                        op=OP.mult)
                nc.vector.tensor_scalar(out=wrow[:], in0=wrow[:],
                                        scalar1=hf[:], scalar2=None,
                                        op0=OP.mult)
                pt4 = pspool.tile([128, 512], f32, tag="pmisc", bufs=2)
                nc.tensor.matmul(pt4[:16, :W], ones1x16[:], wrow[:])
                wrow16 = pp.tile([16, W], f32, tag="wrow16")
                nc.vector.tensor_copy(out=wrow16[:], in_=pt4[0:16, 0:W])
                # weight16 [16,(j w)] = in_h[:,j] * wrow  (bf16)
                for j in range(KJ):
                    nc.vector.tensor_scalar(
                        out=weight16[:, j * W : (j + 1) * W], in0=wrow16[:],
                        scalar1=in_h[:, j : j + 1], scalar2=None, op0=OP.mult)

            def emit_box_mm(i):
                t, j0, j1 = work[i]
                nj = j1 - j0
                sq_i = sq_tiles[i]
                for cl in range((nj * W) // CHUNK):
                    c = (j0 * W) // CHUNK + cl
                    nc.tensor.matmul(
                        box_psum[c][:16, :CHUNK], lhs16[t][:],
                        sq_i[:, cl * CHUNK : (cl + 1) * CHUNK],
                        start=(i == 0), stop=(i == box_stop[c]))

            sq_tiles = []
            for i, (t, j0, j1) in enumerate(work):
                p_t, t_t = bulk_tiles[t]
                fsl = slice(j0 * W, j1 * W)
                nc.vector.tensor_tensor(out=p_t[:, fsl], in0=p_t[:, fsl],
                                        in1=t_t[:, fsl], op=OP.subtract)
                sq_i = sqpool.tile([128, F], bf16, tag=f"sq_{i}")
                nc.scalar.activation(
                    out=sq_i[:, : (j1 - j0) * W], in_=p_t[:, fsl],
                    func=AF.Square, accum_out=acc_tot[:, i : i + 1])
                sq_tiles.append(sq_i)
                if i > SPLIT:
                    emit_box_mm(i)
                if i == SPLIT:
                    emit_cc_dependent()
                    for ii in range(SPLIT + 1):
                        emit_box_mm(ii)

            # ---------------- final reductions ----------------
            junk16 = pp.tile([16, F], bf16, tag="junk16")
            for c in range(4):
                nc.vector.tensor_tensor(
                    out=junk16[:, c * CHUNK : (c + 1) * CHUNK],
                    in0=box_psum[c][0:16, 0:CHUNK],
                    in1=weight16[:, c * CHUNK : (c + 1) * CHUNK],
                    op=OP.mult)
            box_col = pp.tile([16, 1], f32, tag="box_col")
            nc.vector.tensor_reduce(out=box_col[:], in_=junk16[:], axis=AX.X,
                                    op=OP.add)
            tot_col = pp.tile([128, 1], f32, tag="tot_col")
            nc.vector.tensor_reduce(out=tot_col[:], in_=acc_tot[:],
                                    axis=AX.X, op=OP.add)
            pair = pp.tile([128, 2], f32, tag="pair")
            nc.vector.memset(pair[:], 0.0)
            nc.vector.tensor_copy(out=pair[:, 0:1], in_=tot_col[:])
            nc.vector.tensor_copy(out=pair[0:16, 1:2], in_=box_col[:])
            ps_fin = pspool.tile([128, 512], f32, tag="pmisc", bufs=2)
            nc.tensor.matmul(ps_fin[:1, :2], ones128f[:], pair[:])
            res2 = pp.tile([1, 2], f32, tag="res2")
            nc.vector.tensor_copy(out=res2[:], in_=ps_fin[0:1, 0:2])
            nc.scalar.dma_start(
                out=out.ap().rearrange("(p x) -> p x", p=1), in_=res2[:])

    nc.compile()
    return nc


def get_nc():
    if "nc" not in _CACHE:
        _CACHE["nc"] = _build_nc()
    return _CACHE["nc"]


def make_in_maps(y_pred, y_true, mask):
    y_pred = np.asarray(y_pred, dtype=np.float32).reshape(B, D, H, W)
    y_true = np.asarray(y_true, dtype=np.float32).reshape(B, D, H, W)
    mask = np.asarray(mask, dtype=np.int32).reshape(B, D, H, W)
    in_maps = []
    for c in range(N_CORES):
        b, half = c // 2, c % 2
        sl = slice(half * HALF_D, (half + 1) * HALF_D)
        in_maps.append({
            "yp": np.ascontiguousarray(y_pred[b, sl]).reshape(R, W),
            "yt": np.ascontiguousarray(y_true[b, sl]).reshape(R, W),
            "mk": np.ascontiguousarray(mask[b, sl]).reshape(R, W),
            "meta": np.array([half * HALF_D], dtype=np.float32),
        })
    return in_maps


def combine(results):
    tot = 0.0
    box = 0.0
    for r in results:
        o = np.asarray(r["out"], dtype=np.float64).reshape(-1)
        tot += o[0]
        box += o[1]
    loss = (W_OUT2 * tot + (1.0 - W_OUT2) * box) / float(B * D * H * W)
    return np.array(loss, dtype=np.float32)


def kernel(y_pred, y_true, mask):
    from concourse.bass_utils import run_bass_kernel_spmd

    nc = get_nc()
    in_maps = make_in_maps(y_pred, y_true, mask)
    trace = bool(int(os.environ.get("BASS_KERNEL_TRACE", "0")))
    kwargs = {}
    if trace:
        kwargs = dict(trace=True, trace_cores=[0])
    res = run_bass_kernel_spmd(
        nc, in_maps, core_ids=list(range(N_CORES)), **kwargs
    )
    _CACHE["last_results"] = res
    return combine(res.results)
